# revision 2
# baseline (speedup 1.0000x reference)
"""AWQ 4-bit dequant matmul (x[8,4096] @ dequant(qweight)[4096,11008] + bias)
on 8 trn2 NeuronCores, tensor-parallel along the output dim N.

Per core (1376 logical cols): qweight shard streams from HBM; DVE extracts
nibble planes as bf16 bit patterns (v | 0x4180 == 16 + v/8 exactly, so the
dequant needs no int->float conversion); ACT/Pool finish two of the planes;
PE does per-group [128k x 128n x 8b] matmuls with weights stationary (n on
PSUM partitions); the epilogue applies 8*s[g,n] and reduces over the 32
k-groups. Host precomputes fold the zero-points, the +16 offsets (exact,
via per-group sums of the bf16-rounded x), bias, and the AWQ column
permutation; outputs are un-permuted and concatenated on the host.

Self-contained: no imports besides numpy/ml_dtypes/concourse.
"""
import functools
import numpy as np
import ml_dtypes

B, K, N, G = 8, 4096, 11008, 128
NCORES = 8
NG = K // G              # 32 k-groups
NSH = N // NCORES        # 1376 logical cols per core
CSH = NSH // 8           # 172 packed int32 cols per core
NT = 11                  # n-dev tiles of 128 (1408 = padded cols per group)
MPAD = NT * 128
U16PG = 2 * CSH          # 344 u16 elements per group-row
CHUNKS = [8, 8, 8, 8]    # k-group chunks
MAX_WAITS = 1            # walrus in this env: 1 sem-wait per instruction

AWQ_ORDER = np.array([0, 4, 1, 5, 2, 6, 3, 7])


# ---------------------------------------------------------------- tile fixes
def _patch_tile_tail():
    """This walrus build rejects >1 semaphore wait per instruction. Split the
    Tile tail-drain's waits across chained sync-engine NOPs."""
    import concourse.tile as tile
    from concourse.vector_clock import ScopedClock
    from concourse import mybir

    if getattr(tile.TileContext, "_awq_tail_patched", False):
        return

    def _drain_and_barrier(self, tick_clock, wait_clock):
        nc = self.nc
        probe = nc.sync.nop(nofuse=True, hint="tail_wait_probe")
        wait_clock.add_sem_waits(probe.ins,
                                ScopedClock({None: tick_clock.global_clock}))
        waits = list(probe.ins.sync_info.on_wait or [])
        if len(waits) > MAX_WAITS:
            probe.ins.sync_info.on_wait = waits[:MAX_WAITS]
            for i in range(MAX_WAITS, len(waits), MAX_WAITS):
                extra = nc.sync.nop(nofuse=True, hint=f"tail_wait_{i}")
                if extra.ins.sync_info is None:
                    extra.ins.sync_info = mybir.SyncInfo(on_wait=[], on_update=[])
                extra.ins.sync_info.on_wait = waits[i:i + MAX_WAITS]
        nc.sync.drain()
        nc.all_engine_barrier()
        assert self.sems is not None
        popped = nc._tile_sem_poison_stack.pop()
        assert popped is self._sem_poison
        nc.clear_and_free_semaphores(list(self.sems.allocated().values()))
        nc.all_engine_barrier()

    tile.TileContext._drain_and_barrier = _drain_and_barrier
    tile.TileContext._awq_tail_patched = True


def _split_sync_waits(nc):
    """Split any instruction carrying more than MAX_WAITS sem-waits by
    hoisting excess waits onto same-engine NoOps inserted just before it."""
    from concourse import mybir
    for fn in nc.m.functions:
        for blk in fn.blocks:
            out = []
            for inst in blk.instructions:
                si = inst.sync_info
                if si is not None and si.on_wait and len(si.on_wait) > MAX_WAITS:
                    waits = list(si.on_wait)
                    for i in range(0, len(waits) - MAX_WAITS, MAX_WAITS):
                        nop = mybir.InstNoOp(
                            name=nc.get_next_instruction_name(),
                            engine=inst.engine,
                            bass_nofuse=True,
                            sync_info=mybir.SyncInfo(
                                on_wait=waits[i:i + MAX_WAITS], on_update=[]),
                        )
                        nc.register_instruction(nop)
                        out.append(nop)
                    si.on_wait = waits[len(waits) - MAX_WAITS:]
                out.append(inst)
            blk.instructions[:] = out


# ---------------------------------------------------------------- device code
@functools.lru_cache(maxsize=1)
def _build_nc():
    import concourse.bass as bass
    import concourse.tile as tile
    from concourse import mybir
    A = mybir.AluOpType
    dt = mybir.dt
    _patch_tile_tail()

    nc = bass.Bass()
    qs = nc.dram_tensor("qs", [K, CSH], dt.int32, kind="ExternalInput")
    xt = nc.dram_tensor("xt", [128, NG * B], dt.bfloat16, kind="ExternalInput")
    sdev = nc.dram_tensor("sdev", [128, NT * NG], dt.float32, kind="ExternalInput")
    cb = nc.dram_tensor("cb", [128, NT * B], dt.float32, kind="ExternalInput")
    outd = nc.dram_tensor("outd", [128, NT * B], dt.float32, kind="ExternalOutput")

    # per-chunk engine for the two finishing "or" passes: ACT, Pool; the
    # last chunk keeps them on DVE so its slabs are ready sooner.
    OR1 = ["A", "A", "V", "V"]
    OR2 = ["A", "A", "V", "V"]

    with tile.TileContext(nc) as tc:
        with (
            tc.tile_pool(name="const", bufs=1) as cpool,
            tc.tile_pool(name="qp", bufs=1) as qpool,
            tc.tile_pool(name="wp", bufs=1) as wpool,
            tc.tile_pool(name="tmpu", bufs=1) as upool,
            tc.tile_pool(name="ep", bufs=3) as epool,
            tc.tile_pool(name="ps", bufs=1, space="PSUM") as pspool,
        ):
            xt_t = cpool.tile([128, NG * B], dt.bfloat16)
            s_t = cpool.tile([128, NT * NG], dt.float32)
            cb_t = cpool.tile([128, NT * B], dt.float32)
            out_t = cpool.tile([128, NT * B], dt.float32)
            expbias = cpool.tile([128, 1], dt.float32)
            nc.vector.memset(expbias[:], 16768.0)

            psum = [pspool.tile([128, 512], dt.float32, name=f"psum{i}",
                                tag=f"psum{i}") for i in range(6)]

            qsr = qs.rearrange("(g p) c -> p g c", p=128)  # [128, 32, 172]

            def or_pass(dst, srcv, eng):
                if eng == "A":
                    nc.scalar.activation(
                        dst, srcv, mybir.ActivationFunctionType.Identity,
                        bias=expbias[:, 0:1])
                elif eng == "P":
                    nc.gpsimd.tensor_scalar(dst, srcv, 0x4180, None, A.bitwise_or)
                else:
                    nc.vector.tensor_scalar(dst, srcv, 0x4180, None, A.bitwise_or)

            g0 = 0
            for j, GPC in enumerate(CHUNKS):
                qt = qpool.tile([128, GPC * CSH], dt.int32,
                                name=f"qt{j}", tag="qt", bufs=3)
                qv = qt[:].rearrange("p (g c) -> p g c", g=GPC)
                if j == 0:
                    third = GPC // 3 + 1
                    b1, b2 = third, min(2 * third, GPC)
                    nc.sync.dma_start(qv[:, :b1, :], qsr[:, g0:g0 + b1, :])
                    nc.scalar.dma_start(qv[:, b1:b2, :], qsr[:, g0 + b1:g0 + b2, :])
                    nc.gpsimd.dma_start(qv[:, b2:GPC, :], qsr[:, g0 + b2:g0 + GPC, :])
                    nc.sync.dma_start(xt_t[:], xt[:])
                    nc.sync.dma_start(s_t[:], sdev[:])
                    nc.sync.dma_start(cb_t[:], cb[:])
                else:
                    nc.sync.dma_start(qv, qsr[:, g0:g0 + GPC, :])
                wt = wpool.tile([128, GPC * MPAD], dt.uint16,
                                name=f"wt{j}", tag="wt", bufs=3)
                nc.gpsimd.memset(
                    wt[:].rearrange("p (g m) -> p g m", g=GPC)[:, :, 4 * U16PG:], 0)
                u = qt[:].bitcast(dt.uint16)  # [128, GPC*344], u16 stream
                wv = wt[:].rearrange("p (g m) -> p g m", g=GPC)

                def slab(p):
                    return wv[:, :, p * U16PG:(p + 1) * U16PG]

                # Each pass extracts one nibble position of every u16 (two
                # int32 nibble positions via the h0/h1 interleave).
                nc.vector.tensor_scalar(slab(0), u, 0xF, 0x4180,
                                        A.bitwise_and, A.bitwise_or)
                tm = upool.tile([128, GPC * U16PG], dt.uint16,
                                name=f"tmu{j}", tag="tmu", bufs=2)
                nc.vector.tensor_scalar(tm[:], u, 4, 0xF,
                                        A.logical_shift_right, A.bitwise_and)
                or_pass(slab(1), tm[:], OR1[j])
                tm2 = upool.tile([128, GPC * U16PG], dt.uint16,
                                 name=f"tmu2{j}", tag="tmu2", bufs=2)
                nc.vector.tensor_scalar(tm2[:], u, 8, 0xF,
                                        A.logical_shift_right, A.bitwise_and)
                or_pass(slab(2), tm2[:], OR2[j])
                nc.vector.tensor_scalar(slab(3), u, 12, 0x4180,
                                        A.logical_shift_right, A.bitwise_or)

                wb = wt[:].bitcast(dt.bfloat16)
                for t in range(NT):
                    for gl in range(GPC):
                        g = g0 + gl
                        nc.tensor.matmul(
                            psum[t // 2][:, (t % 2) * 256 + g * B:
                                         (t % 2) * 256 + (g + 1) * B],
                            wb[:, gl * MPAD + t * 128: gl * MPAD + (t + 1) * 128],
                            xt_t[:, g * B:(g + 1) * B],
                            start=True, stop=True,
                        )
                g0 += GPC

            for t in range(NT):
                pin = psum[t // 2][:, (t % 2) * 256:(t % 2) * 256 + 256]
                sb = s_t[:, t * NG:(t + 1) * NG, None].broadcast_to([128, NG, B])
                tcp = epool.tile([128, NG * B], dt.float32, name="tcp", tag="tcp")
                nc.scalar.copy(tcp[:], pin)
                tm = epool.tile([128, NG * B], dt.float32, name="etm", tag="etm")
                tv = tm[:].rearrange("p (g b) -> p g b", g=NG)
                nc.gpsimd.tensor_tensor(
                    tv, tcp[:].rearrange("p (g b) -> p g b", g=NG), sb, A.mult)
                w = NG
                while w > 1:
                    h = w // 2
                    nc.gpsimd.tensor_tensor(
                        tv[:, :h, :], tv[:, :h, :], tv[:, h:w, :], A.add)
                    w = h
                nc.gpsimd.tensor_tensor(
                    out_t[:, t * B:(t + 1) * B], tv[:, 0, :],
                    cb_t[:, t * B:(t + 1) * B], A.add)
            nc.sync.dma_start(outd[:], out_t[:])

    _split_sync_waits(nc)
    return nc


# ---------------------------------------------------------------- host side
def _unpack_awq_np(q):
    shifts = AWQ_ORDER * 4
    u = (q[:, :, None].view(np.uint32) >> shifts[None, None, :]) & 0xF
    return u.reshape(q.shape[0], -1).astype(np.int32)


def _mdev_to_nlocal():
    m = np.arange(MPAD)
    valid = m < 4 * U16PG
    p = np.clip(m, 0, 4 * U16PG - 1) // U16PG
    i = np.clip(m, 0, 4 * U16PG - 1) % U16PG
    n = 8 * (i // 2) + 2 * p + (i % 2)
    return np.where(valid, n, -1)


MDEV2NLOC = _mdev_to_nlocal()


def _host_prepare(x, qweight, scales, qzeros, bias):
    xbf = x.astype(ml_dtypes.bfloat16)
    xf = xbf.astype(np.float32)
    t_g = xf.reshape(B, NG, G).astype(np.float64).sum(axis=2).T  # [NG, B]

    iz = _unpack_awq_np(qzeros)
    zc = scales.astype(np.float64) * (iz + 128.0)
    corr = np.einsum("gb,gn->bn", t_g, zc)
    cb_full = bias.astype(np.float64)[None, :] - corr            # [B, N]

    xt = np.ascontiguousarray(
        xbf.reshape(B, NG, G).transpose(2, 1, 0)).reshape(128, NG * B)

    m = np.arange(MPAD)
    t_idx = m // 128
    p_idx = m % 128
    valid = MDEV2NLOC >= 0

    in_maps = []
    for r in range(NCORES):
        nglob = np.where(valid, MDEV2NLOC + r * NSH, 0)
        sv = (8.0 * scales[:, nglob]).astype(np.float32)
        sv[:, ~valid] = 0.0
        sdev_r = np.zeros((128, NT, NG), np.float32)
        sdev_r[p_idx, t_idx, :] = sv.T
        cbv = cb_full[:, nglob].astype(np.float32)
        cbv[:, ~valid] = 0.0
        cb_r = np.zeros((128, NT, B), np.float32)
        cb_r[p_idx, t_idx, :] = cbv.T
        in_maps.append({
            "qs": np.ascontiguousarray(qweight[:, r * CSH:(r + 1) * CSH]),
            "xt": xt,
            "sdev": sdev_r.reshape(128, NT * NG),
            "cb": cb_r.reshape(128, NT * B),
        })
    return in_maps


def _host_gather(results):
    out = np.empty((B, N), np.float32)
    m = np.arange(MPAD)
    t_idx = m // 128
    p_idx = m % 128
    valid = MDEV2NLOC >= 0
    for r in range(NCORES):
        od = np.asarray(results[r]["outd"]).reshape(128, NT, B)
        vals = od[p_idx, t_idx, :]
        out[:, MDEV2NLOC[valid] + r * NSH] = vals[valid].T
    return out


def kernel(x, qweight, scales, qzeros, bias, group_size):
    assert int(group_size) == G
    x = np.asarray(x, dtype=np.float32)
    qweight = np.asarray(qweight, dtype=np.int32)
    scales = np.asarray(scales, dtype=np.float32)
    qzeros = np.asarray(qzeros, dtype=np.int32)
    bias = np.asarray(bias, dtype=np.float32)
    assert x.shape == (B, K) and qweight.shape == (K, N // 8)

    from concourse.bass_utils import run_bass_kernel_spmd
    nc = _build_nc()
    in_maps = _host_prepare(x, qweight, scales, qzeros, bias)
    res = run_bass_kernel_spmd(nc, in_maps, list(range(NCORES)))
    return _host_gather(res.results)
